# revision 7
# baseline (speedup 1.0000x reference)
"""Trainium2 Bass kernel for an 8-layer decoder-only transformer.

Model: B=4, S=1024, D=512, H=8 heads (dk=64), DFF=2048, 8 layers, pre-LN,
causal attention, final LN.  Reference semantics (jax fp32):
  x += sinusoidal PE
  per layer:  h=LN1(x); q,k,v = h@W?+b?; attn = causal-softmax(q k^T/8) v
              x += attn@Wo+bo;  h2=LN2(x);  x += relu(h2@W1+b1)@W2+b2
  out = LNf(x)     (LN uses Bessel std, eps added to std)

Sharding: sequence-parallel over 8 NeuronCores.  Core r owns the 128
sequence positions [r*128,(r+1)*128) for all 4 batches (512 rows).  All
projections / LN / FFN are row-parallel with replicated weights.  The only
communication is one AllGather of (K, V) per layer.  Every core computes
scores against all 1024 key positions; a per-core 0/1 mask (kernel input)
zeroes invisible keys, keeping one SPMD program for all cores.

On-device layout: activations are kept feature-major ("transposed",
[D partitions x rows free]) so matmuls chain without transposes.
V is produced row-major (activation-stationary matmul) with an extra
"ones" column per head so the attention row-sum (softmax denominator)
falls out of the attention*V matmul for free.
"""

import math
import sys

import numpy as np

for _p in ("/root/.axon_site/_ro/trn_rl_repo", "/opt/trn_rl_repo"):
    if _p not in sys.path:
        sys.path.append(_p)

import ml_dtypes  # noqa: E402
import concourse.bass as bass  # noqa: E402
import concourse.bacc as bacc  # noqa: E402
import concourse.tile as tile  # noqa: E402
import concourse.mybir as mybir  # noqa: E402
from concourse import bass_utils  # noqa: E402

# Model dims (hardcoded per the problem spec)
B, S, D, H, DFF, NL = 4, 1024, 512, 8, 2048, 8
DK = D // H              # 64
NCORES = 8
SC = S // NCORES         # 128 seq positions per core
R = B * SC               # 512 rows per core (row = b*SC + s_local)
P = 128
NT = D // P              # 4 feature tiles
EPS = 1e-6

F32 = mybir.dt.float32
F32R = mybir.dt.float32r
BF16 = mybir.dt.bfloat16
AF = mybir.ActivationFunctionType
OP = mybir.AluOpType
BF16NP = ml_dtypes.bfloat16

VW = H * (DK + 1)        # 520: V row-major width with per-head ones column


def _build():
    nc = bacc.Bacc(
        "TRN2", target_bir_lowering=False, debug=False, num_devices=NCORES
    )

    # ---- kernel I/O -------------------------------------------------------
    x_t = nc.dram_tensor("x_t", [D, R], F32, kind="ExternalInput")
    pe_t = nc.dram_tensor("pe_t", [D, SC], F32, kind="ExternalInput")
    mask_d = nc.dram_tensor("mask", [P, S], BF16, kind="ExternalInput")
    wq_d = nc.dram_tensor("wq", [NL, D, D], BF16, kind="ExternalInput")
    wk_d = nc.dram_tensor("wk", [NL, D, D], BF16, kind="ExternalInput")
    wv_d = nc.dram_tensor("wv", [NL, D, D], BF16, kind="ExternalInput")
    wo_d = nc.dram_tensor("wo", [NL, D, D], BF16, kind="ExternalInput")
    w1_d = nc.dram_tensor("w1", [NL, D, DFF], BF16, kind="ExternalInput")
    w2_d = nc.dram_tensor("w2", [NL, DFF, D], BF16, kind="ExternalInput")
    bq_d = nc.dram_tensor("bq", [NL, D], F32, kind="ExternalInput")
    bk_d = nc.dram_tensor("bk", [NL, D], F32, kind="ExternalInput")
    bv_d = nc.dram_tensor("bv", [NL, D], F32, kind="ExternalInput")
    bo_d = nc.dram_tensor("bo", [NL, D], F32, kind="ExternalInput")
    b1_d = nc.dram_tensor("b1", [NL, DFF], F32, kind="ExternalInput")
    b2_d = nc.dram_tensor("b2", [NL, D], F32, kind="ExternalInput")
    l1a_d = nc.dram_tensor("l1a", [NL, D], F32, kind="ExternalInput")
    l1b_d = nc.dram_tensor("l1b", [NL, D], F32, kind="ExternalInput")
    l2a_d = nc.dram_tensor("l2a", [NL, D], F32, kind="ExternalInput")
    l2b_d = nc.dram_tensor("l2b", [NL, D], F32, kind="ExternalInput")
    fa_d = nc.dram_tensor("fa", [D], F32, kind="ExternalInput")
    fb_d = nc.dram_tensor("fb", [D], F32, kind="ExternalInput")
    y_d = nc.dram_tensor("y_t", [D, R], F32, kind="ExternalOutput")

    with tile.TileContext(nc) as tc:
        with tc.tile_pool(name="sp", bufs=1) as sp, \
             tc.tile_pool(name="pp", bufs=1, space="PSUM") as pp, \
             tc.tile_pool(name="dp", bufs=1, space="DRAM") as dp:

            def f32r(ap):
                return ap.bitcast(F32R)

            # ---- constants ------------------------------------------------
            ones_col = sp.tile([P, 1], BF16, tag="const", bufs=4, name="ones_col")
            nc.vector.memset(ones_col[:], 1.0)
            ones_row_bf = sp.tile([1, D], BF16, tag="const", bufs=4,
                                  name="ones_row_bf")
            nc.vector.memset(ones_row_bf[:], 1.0)
            mask_sb = sp.tile([P, S], BF16, tag="mask", bufs=1, name="mask_sb")
            nc.sync.dma_start(mask_sb[:], mask_d.ap())

            def load_cols(name, src_ap, d):
                # [d] f32 dram vector -> SBUF [P, d//P] (col t = slice t)
                t_ = sp.tile([P, d // P], F32, tag="bias", bufs=24, name=name)
                nc.sync.dma_start(
                    t_[:], src_ap.rearrange("(t p) -> p t", p=P)
                )
                return t_

            fa_col = load_cols("fa_col", fa_d.ap(), D)
            fb_col = load_cols("fb_col", fb_d.ap(), D)

            # ---- layer-0 input: x slice + positional encoding -------------
            xt = []
            for t in range(NT):
                xtile = sp.tile([P, R], F32, tag="xt", bufs=8, name="xtile")
                nc.sync.dma_start(xtile[:], x_t.ap()[t * P:(t + 1) * P, :])
                xt.append(xtile)
            pe_sb = []
            for t in range(NT):
                ptile = sp.tile([P, SC], F32, tag="pe", bufs=4, name="ptile")
                nc.sync.dma_start(ptile[:], pe_t.ap()[t * P:(t + 1) * P, :])
                pe_sb.append(ptile)
            for t in range(NT):
                for b in range(B):
                    nc.vector.tensor_add(
                        xt[t][:, b * SC:(b + 1) * SC], xt[t][:, b * SC:(b + 1) * SC],
                        pe_sb[t][:],
                    )

            # ---- helpers --------------------------------------------------
            def layernorm(xts, a_col, b_col, out_dtype, out_tag, out_bufs):
                """Feature-major LN over the partition axis (4 tiles)."""
                sum_ps = pp.tile([1, R], F32, tag="small", bufs=2, name="sum_ps")
                for t in range(NT):
                    xb = sp.tile([P, R], BF16, tag="xb", bufs=3, name="xb")
                    nc.vector.tensor_copy(xb[:], xts[t][:])
                    nc.tensor.matmul(
                        sum_ps[:], ones_col[:], xb[:],
                        start=(t == 0), stop=(t == NT - 1),
                    )
                ssq_ps = pp.tile([1, R], F32, tag="small", bufs=2, name="ssq_ps")
                for t in range(NT):
                    sq = sp.tile([P, R], BF16, tag="xb", bufs=3, name="sq")
                    nc.vector.tensor_mul(sq[:], xts[t][:], xts[t][:])
                    nc.tensor.matmul(
                        ssq_ps[:], ones_col[:], sq[:],
                        start=(t == 0), stop=(t == NT - 1),
                    )
                mean_sb = sp.tile([1, R], F32, tag="stat", bufs=4, name="mean_sb")
                nc.vector.tensor_scalar_mul(mean_sb[:], sum_ps[:], 1.0 / D)
                t1 = sp.tile([1, R], F32, tag="stat", bufs=4, name="t1")
                nc.vector.tensor_mul(t1[:], sum_ps[:], mean_sb[:])
                t2 = sp.tile([1, R], F32, tag="stat", bufs=4, name="t2")
                nc.vector.tensor_sub(t2[:], ssq_ps[:], t1[:])
                # std = sqrt(t2/(D-1)); denom = std + eps; inv = 1/denom
                std = sp.tile([1, R], F32, tag="stat", bufs=4, name="std")
                nc.scalar.activation(std[:], t2[:], AF.Sqrt, scale=1.0 / (D - 1))
                se = sp.tile([1, R], F32, tag="stat", bufs=4, name="se")
                nc.vector.tensor_scalar_add(se[:], std[:], EPS)
                inv = sp.tile([1, R], F32, tag="stat", bufs=4, name="inv")
                nc.vector.reciprocal(inv[:], se[:])
                m_b = sp.tile([P, R], F32, tag="bcast", bufs=4, name="m_b")
                nc.gpsimd.partition_broadcast(m_b[:], mean_sb[:], channels=P)
                inv_b = sp.tile([P, R], F32, tag="bcast", bufs=4, name="inv_b")
                nc.gpsimd.partition_broadcast(inv_b[:], inv[:], channels=P)
                outs = []
                for t in range(NT):
                    tt = sp.tile([P, R], F32, tag="tmp", bufs=3, name="tt")
                    nc.vector.tensor_sub(tt[:], xts[t][:], m_b[:])
                    tt2 = sp.tile([P, R], F32, tag="tmp", bufs=3, name="tt2")
                    nc.vector.scalar_tensor_tensor(
                        tt2[:], tt[:], a_col[:, t:t + 1], inv_b[:],
                        op0=OP.mult, op1=OP.mult,
                    )
                    o = sp.tile([P, R], out_dtype, tag=out_tag, bufs=out_bufs,
                                name="lnout")
                    nc.vector.tensor_scalar_add(o[:], tt2[:], b_col[:, t:t + 1])
                    outs.append(o)
                return outs

            def proj_w_stationary(h_tiles, w_tiles, bias_col, out_tag,
                                  out_bufs, out_dtype=BF16):
                """out[dout, rows] (transposed): lhsT = W[kt][:, mtile]."""
                outs = []
                for m in range(NT):
                    ps = pp.tile([P, R], F32, tag="mm", bufs=2, name="proj_ps")
                    for k in range(NT):
                        nc.tensor.matmul(
                            ps[:], w_tiles[k][:, m * P:(m + 1) * P],
                            h_tiles[k][:], start=(k == 0), stop=(k == NT - 1),
                        )
                    o = sp.tile([P, R], out_dtype, tag=out_tag, bufs=out_bufs,
                                name="proj_o")
                    nc.vector.tensor_scalar_add(o[:], ps[:], bias_col[:, m:m + 1])
                    outs.append(o)
                return outs

            # ---- the 8 layers --------------------------------------------
            for l in range(NL):
                # per-layer bias / LN param columns
                bq_c = load_cols("bq_c", bq_d.ap()[l], D)
                bk_c = load_cols("bk_c", bk_d.ap()[l], D)
                bo_c = load_cols("bo_c", bo_d.ap()[l], D)
                b1_c = load_cols("b1_c", b1_d.ap()[l], DFF)
                b2_c = load_cols("b2_c", b2_d.ap()[l], D)
                l1a_c = load_cols("l1a_c", l1a_d.ap()[l], D)
                l1b_c = load_cols("l1b_c", l1b_d.ap()[l], D)
                l2a_c = load_cols("l2a_c", l2a_d.ap()[l], D)
                l2b_c = load_cols("l2b_c", l2b_d.ap()[l], D)
                bv_row = sp.tile([1, D], F32, tag="bvrow", bufs=2, name="bv_row")
                nc.sync.dma_start(
                    bv_row[:], bv_d.ap()[l].rearrange("(o d) -> o d", o=1)
                )
                bv_rowb = sp.tile([1, D], BF16, tag="bvrow", bufs=2,
                                  name="bv_rowb")
                nc.vector.tensor_copy(bv_rowb[:], bv_row[:])

                # weight tiles
                wq_sb = [sp.tile([P, D], BF16, tag="wq", bufs=4, name="wq_sb")
                         for _ in range(NT)]
                wk_sb = [sp.tile([P, D], BF16, tag="wk", bufs=4, name="wk_sb")
                         for _ in range(NT)]
                wv_sb = [sp.tile([P, D], BF16, tag="wv", bufs=4, name="wv_sb")
                         for _ in range(NT)]
                wo_sb = [sp.tile([P, D], BF16, tag="wo", bufs=4, name="wo_sb")
                         for _ in range(NT)]
                w1_sb = [sp.tile([P, DFF], BF16, tag="w1", bufs=4, name="w1_sb")
                         for _ in range(NT)]
                w2_sb = [sp.tile([P, D], BF16, tag="w2", bufs=17, name="w2_sb")
                         for _ in range(DFF // P)]
                for k in range(NT):
                    nc.sync.dma_start(wk_sb[k][:], wk_d.ap()[l, k * P:(k + 1) * P, :])
                    nc.sync.dma_start(wv_sb[k][:], wv_d.ap()[l, k * P:(k + 1) * P, :])
                    nc.sync.dma_start(wq_sb[k][:], wq_d.ap()[l, k * P:(k + 1) * P, :])
                    nc.sync.dma_start(wo_sb[k][:], wo_d.ap()[l, k * P:(k + 1) * P, :])
                    nc.sync.dma_start(w1_sb[k][:], w1_d.ap()[l, k * P:(k + 1) * P, :])
                for k in range(DFF // P):
                    nc.sync.dma_start(w2_sb[k][:], w2_d.ap()[l, k * P:(k + 1) * P, :])

                # LN1 -> h1 (bf16, feature-major)
                h1 = layernorm(xt, l1a_c, l1b_c, BF16, "h1", 4)

                # K projection (transposed) and V projection (row-major,
                # with per-head ones column) -> AllGather bounce
                agin = dp.tile([2 * R, VW], BF16, tag="agin", bufs=2,
                               name="agin")
                kt_sb = proj_w_stationary(h1, wk_sb, bk_c, "kt", 4)
                for t in range(NT):
                    nc.sync.dma_start(
                        agin[t * P:(t + 1) * P, 0:D], kt_sb[t][:]
                    )
                for rt in range(NT):
                    vps = pp.tile([P, D], F32, tag="mm", bufs=2, name="vps")
                    nc.tensor.matmul(
                        vps[:], ones_row_bf[:, 0:P], bv_rowb[:],
                        start=True, stop=False,
                    )
                    for k in range(NT):
                        nc.tensor.matmul(
                            vps[:], h1[k][:, rt * P:(rt + 1) * P], wv_sb[k][:],
                            start=False, stop=(k == NT - 1),
                        )
                    v_sb = sp.tile([P, VW], BF16, tag="v_sb", bufs=3,
                                   name="v_sb")
                    nc.vector.memset(v_sb[:], 1.0)
                    nc.scalar.activation(
                        v_sb[:].rearrange("p (h w) -> p h w", w=DK + 1)[:, :, 0:DK],
                        vps[:].rearrange("p (h w) -> p h w", w=DK),
                        AF.Copy,
                    )
                    nc.sync.dma_start(
                        agin[R + rt * P:R + (rt + 1) * P, :], v_sb[:]
                    )
                agout = dp.tile([NCORES * 2 * R, VW], BF16, tag="agout",
                                bufs=2, addr_space="Shared", name="agout")
                nc.gpsimd.collective_compute(
                    "AllGather", OP.bypass,
                    replica_groups=[list(range(NCORES))],
                    ins=[agin.opt()], outs=[agout.opt()],
                )

                # Q projection (overlaps the AllGather)
                qt_sb = proj_w_stationary(h1, wq_sb, bq_c, "qt", 4)

                # load gathered K into SBUF (resident for the layer)
                kreg = {}
                for rc in range(NCORES):
                    for t in range(NT):
                        kr = sp.tile([P, R], BF16, tag="kreg", bufs=32,
                                     name="kr")
                        nc.sync.dma_start(
                            kr[:],
                            agout[rc * 2 * R + t * P: rc * 2 * R + (t + 1) * P,
                                  0:D],
                        )
                        kreg[(rc, t)] = kr

                # attention: batch-outer so V tiles stream per batch
                atn = [sp.tile([P, R], BF16, tag="atn", bufs=4, name="atn")
                       for _ in range(NT)]
                for b in range(B):
                    vreg = []
                    for rc in range(NCORES):
                        vr = sp.tile([P, VW], BF16, tag="vreg", bufs=10,
                                     name="vr")
                        nc.sync.dma_start(
                            vr[:],
                            agout[rc * 2 * R + R + b * P:
                                  rc * 2 * R + R + (b + 1) * P, :],
                        )
                        vreg.append(vr)
                    for h in range(H):
                        t_h, o_h = h // 2, (h % 2) * DK
                        sc_ps = pp.tile([P, S], F32, tag="scores", bufs=2,
                                        name="sc_ps")
                        for kc in range(NCORES):
                            nc.tensor.matmul(
                                sc_ps[:, kc * P:(kc + 1) * P],
                                kreg[(kc, t_h)][o_h:o_h + DK, b * SC:(b + 1) * SC],
                                qt_sb[t_h][o_h:o_h + DK, b * SC:(b + 1) * SC],
                                start=True, stop=True,
                            )
                        probs = sp.tile([P, S], BF16, tag="probs", bufs=2,
                                        name="probs")
                        nc.scalar.activation(
                            probs[:], sc_ps[:], AF.Exp,
                            scale=1.0 / math.sqrt(DK),
                        )
                        nc.vector.tensor_mul(probs[:], probs[:], mask_sb[:])
                        at_ps = pp.tile([DK + 1, SC], F32, tag="small", bufs=2,
                                        name="at_ps")
                        for kc in range(NCORES):
                            nc.tensor.matmul(
                                at_ps[:],
                                vreg[kc][:, h * (DK + 1):(h + 1) * (DK + 1)],
                                probs[:, kc * P:(kc + 1) * P],
                                start=(kc == 0), stop=(kc == NCORES - 1),
                            )
                        recip = sp.tile([1, SC], F32, tag="stat", bufs=4,
                                        name="recip")
                        nc.vector.reciprocal(recip[:], at_ps[DK:DK + 1, :])
                        rb_sb = sp.tile([DK, SC], F32, tag="rb", bufs=3,
                                        name="rb_sb")
                        nc.gpsimd.partition_broadcast(
                            rb_sb[:], recip[:], channels=DK
                        )
                        nc.vector.tensor_mul(
                            atn[t_h][o_h:o_h + DK, b * SC:(b + 1) * SC],
                            at_ps[0:DK, :], rb_sb[:],
                        )

                # Wo projection (transposed out) + residual
                xt2 = []
                for m in range(NT):
                    ops = pp.tile([P, R], F32, tag="mm", bufs=2, name="ops")
                    for k in range(NT):
                        nc.tensor.matmul(
                            ops[:], wo_sb[k][:, m * P:(m + 1) * P], atn[k][:],
                            start=(k == 0), stop=(k == NT - 1),
                        )
                    xn = sp.tile([P, R], F32, tag="xt", bufs=8, name="xn")
                    nc.vector.scalar_tensor_tensor(
                        xn[:], ops[:], bo_c[:, m:m + 1], xt[m][:],
                        op0=OP.add, op1=OP.add,
                    )
                    xt2.append(xn)

                # LN2 -> h2; FFN
                h2 = layernorm(xt2, l2a_c, l2b_c, BF16, "h1", 4)
                zt = []
                for mt in range(DFF // P):
                    zps = pp.tile([P, R], F32, tag="mm", bufs=2, name="zps")
                    for k in range(NT):
                        nc.tensor.matmul(
                            zps[:], w1_sb[k][:, mt * P:(mt + 1) * P], h2[k][:],
                            start=(k == 0), stop=(k == NT - 1),
                        )
                    z = sp.tile([P, R], BF16, tag="zt", bufs=17, name="z")
                    nc.scalar.activation(
                        z[:], zps[:], AF.Relu, bias=b1_c[:, mt:mt + 1]
                    )
                    zt.append(z)
                xt3 = []
                for m in range(NT):
                    fps = pp.tile([P, R], F32, tag="mm", bufs=2, name="fps")
                    for k in range(DFF // P):
                        nc.tensor.matmul(
                            fps[:], w2_sb[k][:, m * P:(m + 1) * P], zt[k][:],
                            start=(k == 0), stop=(k == DFF // P - 1),
                        )
                    xn2 = sp.tile([P, R], F32, tag="xt", bufs=8, name="xn2")
                    nc.vector.scalar_tensor_tensor(
                        xn2[:], fps[:], b2_c[:, m:m + 1], xt2[m][:],
                        op0=OP.add, op1=OP.add,
                    )
                    xt3.append(xn2)
                xt = xt3

            # ---- final LN + output ---------------------------------------
            yt = layernorm(xt, fa_col, fb_col, F32, "yt", 5)
            for t in range(NT):
                nc.sync.dma_start(y_d.ap()[t * P:(t + 1) * P, :], yt[t][:])

    nc.compile()
    return nc


_CACHED = {}


def _get_program():
    if "nc" not in _CACHED:
        _CACHED["nc"] = _build()
    return _CACHED["nc"]


def _pe_table():
    pos = np.arange(S, dtype=np.float32)[:, None]
    div = np.exp(
        np.arange(0, D, 2, dtype=np.float32) * (-math.log(10000.0) / D)
    )
    pe = np.zeros((S, D), np.float32)
    pe[:, 0::2] = np.sin(pos * div)
    pe[:, 1::2] = np.cos(pos * div)
    return pe


def _make_in_maps(inputs):
    f32 = lambda a: np.ascontiguousarray(np.asarray(a), dtype=np.float32)
    bf = lambda a: np.ascontiguousarray(np.asarray(a)).astype(BF16NP)

    x = f32(inputs["x"])                      # (B, S, D)
    pe = _pe_table()
    shared = {
        "wq": bf(inputs["Wq"]), "wk": bf(inputs["Wk"]),
        "wv": bf(inputs["Wv"]), "wo": bf(inputs["Wo"]),
        "w1": bf(inputs["W1"]), "w2": bf(inputs["W2"]),
        "bq": f32(inputs["bq"]), "bk": f32(inputs["bk"]),
        "bv": f32(inputs["bv"]), "bo": f32(inputs["bo"]),
        "b1": f32(inputs["b1"]), "b2": f32(inputs["b2"]),
        "l1a": f32(inputs["ln1_a"]), "l1b": f32(inputs["ln1_b"]),
        "l2a": f32(inputs["ln2_a"]), "l2b": f32(inputs["ln2_b"]),
        "fa": f32(inputs["fa"]), "fb": f32(inputs["fb"]),
    }
    in_maps = []
    for r in range(NCORES):
        sl = slice(r * SC, (r + 1) * SC)
        # [D, B*SC] feature-major slice, rows b-major
        x_t = np.ascontiguousarray(
            x[:, sl, :].transpose(2, 0, 1).reshape(D, R)
        )
        pe_t = np.ascontiguousarray(pe[sl, :].T)
        # mask[k, kc*128+q] = 1 if key (kc*128+k) is visible to query (r*128+q)
        kglob = (np.arange(NCORES)[:, None, None] * P
                 + np.arange(P)[None, :, None])          # [kc, k, 1]
        qglob = r * P + np.arange(P)[None, None, :]       # [1, 1, q]
        m = (kglob <= qglob)                              # [kc, k, q]
        mask = np.ascontiguousarray(
            m.transpose(1, 0, 2).reshape(P, S)
        ).astype(BF16NP)
        in_maps.append({"x_t": x_t, "pe_t": pe_t, "mask": mask, **shared})
    return in_maps


def kernel(**inputs):
    nc = _get_program()
    in_maps = _make_in_maps(inputs)
    res = bass_utils.run_bass_kernel_spmd(
        nc, in_maps, core_ids=list(range(NCORES))
    )
    out = np.empty((B, S, D), np.float32)
    for r in range(NCORES):
        y_t = res.results[r]["y_t"]                       # [D, R]
        out[:, r * SC:(r + 1) * SC, :] = (
            y_t.reshape(D, B, SC).transpose(1, 2, 0)
        )
    return out


if __name__ == "__main__":
    rng = np.random.default_rng(0)
    ins = {"x": rng.standard_normal((B, S, D)).astype(np.float32)}
    print(kernel(**ins).shape)


# revision 20
# speedup vs baseline: 1.1206x; 1.1206x over previous
"""Trainium2 Bass kernel for an 8-layer decoder-only transformer.

Model: B=4, S=1024, D=512, H=8 heads (dk=64), DFF=2048, 8 layers, pre-LN,
causal attention, final LN.  Reference semantics (jax fp32):
  x += sinusoidal PE
  per layer:  h=LN1(x); q,k,v = h@W?+b?; attn = causal-softmax(q k^T/8) v
              x += attn@Wo+bo;  h2=LN2(x);  x += relu(h2@W1+b1)@W2+b2
  out = LNf(x)     (LN uses Bessel std, eps added to std)

Sharding: sequence-parallel over 8 NeuronCores.  Core r owns the 128
sequence positions [r*128,(r+1)*128) for all 4 batches (512 rows).  All
projections / LN / FFN are row-parallel with replicated weights.  The only
communication is one AllGather of (K, V) per layer.  Every core computes
scores against all 1024 key positions; a per-core 0/1 mask (kernel input)
zeroes invisible keys, keeping one SPMD program for all cores.

On-device layout: activations are kept feature-major ("transposed",
[D partitions x rows free]) so matmuls chain without transposes.
V is produced row-major (activation-stationary matmul) with an extra
"ones" column per head so the attention row-sum (softmax denominator)
falls out of the attention*V matmul for free.
"""

import math
import sys

import numpy as np

for _p in ("/root/.axon_site/_ro/trn_rl_repo", "/opt/trn_rl_repo"):
    if _p not in sys.path:
        sys.path.append(_p)

import ml_dtypes  # noqa: E402
import concourse.bass as bass  # noqa: E402
import concourse.bacc as bacc  # noqa: E402
import concourse.tile as tile  # noqa: E402
import concourse.mybir as mybir  # noqa: E402
from concourse import bass_utils  # noqa: E402

# Model dims (hardcoded per the problem spec)
B, S, D, H, DFF, NL = 4, 1024, 512, 8, 2048, 8
DK = D // H              # 64
NCORES = 8
SC = S // NCORES         # 128 seq positions per core
R = B * SC               # 512 rows per core (row = b*SC + s_local)
P = 128
NT = D // P              # 4 feature tiles
EPS = 1e-6

F32 = mybir.dt.float32
F32R = mybir.dt.float32r
BF16 = mybir.dt.bfloat16
AF = mybir.ActivationFunctionType
OP = mybir.AluOpType
BF16NP = ml_dtypes.bfloat16

VW = H * (DK + 1)        # 520: V row-major width with per-head ones column


def _build():
    nc = bacc.Bacc(
        "TRN2", target_bir_lowering=False, debug=False, num_devices=NCORES
    )

    # ---- kernel I/O -------------------------------------------------------
    x_t = nc.dram_tensor("x_t", [D, R], F32, kind="ExternalInput")
    pe_t = nc.dram_tensor("pe_t", [D, SC], F32, kind="ExternalInput")
    mask_d = nc.dram_tensor("mask", [P, S], BF16, kind="ExternalInput")
    wq_d = nc.dram_tensor("wq", [NL, D, D], BF16, kind="ExternalInput")
    wk_d = nc.dram_tensor("wk", [NL, D, D], BF16, kind="ExternalInput")
    wv_d = nc.dram_tensor("wv", [NL, D, D], BF16, kind="ExternalInput")
    wo_d = nc.dram_tensor("wo", [NL, D, D], BF16, kind="ExternalInput")
    w1_d = nc.dram_tensor("w1", [NL, D, DFF], BF16, kind="ExternalInput")
    w2_d = nc.dram_tensor("w2", [NL, DFF, D], BF16, kind="ExternalInput")
    bq_d = nc.dram_tensor("bq", [NL, D], F32, kind="ExternalInput")
    bk_d = nc.dram_tensor("bk", [NL, D], F32, kind="ExternalInput")
    bv_d = nc.dram_tensor("bv", [NL, D], F32, kind="ExternalInput")
    bo_d = nc.dram_tensor("bo", [NL, D], F32, kind="ExternalInput")
    b1_d = nc.dram_tensor("b1", [NL, DFF], F32, kind="ExternalInput")
    b2_d = nc.dram_tensor("b2", [NL, D], F32, kind="ExternalInput")
    l1a_d = nc.dram_tensor("l1a", [NL, D], F32, kind="ExternalInput")
    l1b_d = nc.dram_tensor("l1b", [NL, D], F32, kind="ExternalInput")
    l2a_d = nc.dram_tensor("l2a", [NL, D], F32, kind="ExternalInput")
    l2b_d = nc.dram_tensor("l2b", [NL, D], F32, kind="ExternalInput")
    fa_d = nc.dram_tensor("fa", [D], F32, kind="ExternalInput")
    fb_d = nc.dram_tensor("fb", [D], F32, kind="ExternalInput")
    y_d = nc.dram_tensor("y_t", [D, R], F32, kind="ExternalOutput")

    with tile.TileContext(nc) as tc:
        with tc.tile_pool(name="sp", bufs=1) as sp, \
             tc.tile_pool(name="pp", bufs=1, space="PSUM") as pp, \
             tc.tile_pool(name="dp", bufs=1, space="DRAM") as dp:

            def f32r(ap):
                return ap.bitcast(F32R)

            # ---- constants ------------------------------------------------
            ones_col = sp.tile([P, 1], BF16, tag="const", bufs=4, name="ones_col")
            nc.vector.memset(ones_col[:], 1.0)
            ones_row_bf = sp.tile([1, D], BF16, tag="const", bufs=4,
                                  name="ones_row_bf")
            nc.vector.memset(ones_row_bf[:], 1.0)
            mask_sb = sp.tile([P, S], BF16, tag="mask", bufs=1, name="mask_sb")
            nc.sync.dma_start(mask_sb[:], mask_d.ap())

            def load_cols(name, src_ap, d):
                # [d] f32 dram vector -> SBUF [P, d//P] (col t = slice t)
                t_ = sp.tile([P, d // P], F32, tag="bias", bufs=24, name=name)
                nc.sync.dma_start(
                    t_[:], src_ap.rearrange("(t p) -> p t", p=P)
                )
                return t_

            fa_col = load_cols("fa_col", fa_d.ap(), D)
            fb_col = load_cols("fb_col", fb_d.ap(), D)

            # ---- layer-0 input: x slice + positional encoding -------------
            xt = []
            for t in range(NT):
                xtile = sp.tile([P, R], F32, tag="xt", bufs=8, name="xtile")
                nc.sync.dma_start(xtile[:], x_t.ap()[t * P:(t + 1) * P, :])
                xt.append(xtile)
            pe_sb = []
            for t in range(NT):
                ptile = sp.tile([P, SC], F32, tag="pe", bufs=4, name="ptile")
                nc.sync.dma_start(ptile[:], pe_t.ap()[t * P:(t + 1) * P, :])
                pe_sb.append(ptile)
            for t in range(NT):
                for b in range(B):
                    nc.vector.tensor_add(
                        xt[t][:, b * SC:(b + 1) * SC], xt[t][:, b * SC:(b + 1) * SC],
                        pe_sb[t][:],
                    )

            # ---- helpers --------------------------------------------------
            def layernorm(xts, a_col, b_col, out_dtype, out_tag, out_bufs):
                """Feature-major LN over the partition axis (4 tiles)."""
                sum_ps = pp.tile([1, R], F32, tag="small", bufs=2, name="sum_ps")
                for t in range(NT):
                    xb = sp.tile([P, R], BF16, tag="xb", bufs=3, name="xb")
                    nc.vector.tensor_copy(xb[:], xts[t][:])
                    nc.tensor.matmul(
                        sum_ps[:], ones_col[:], xb[:],
                        start=(t == 0), stop=(t == NT - 1),
                    )
                ssq_ps = pp.tile([1, R], F32, tag="small", bufs=2, name="ssq_ps")
                for t in range(NT):
                    sq = sp.tile([P, R], BF16, tag="xb", bufs=3, name="sq")
                    nc.vector.tensor_mul(sq[:], xts[t][:], xts[t][:])
                    nc.tensor.matmul(
                        ssq_ps[:], ones_col[:], sq[:],
                        start=(t == 0), stop=(t == NT - 1),
                    )
                mean_sb = sp.tile([1, R], F32, tag="stat", bufs=4, name="mean_sb")
                nc.vector.tensor_scalar_mul(mean_sb[:], sum_ps[:], 1.0 / D)
                t1 = sp.tile([1, R], F32, tag="stat", bufs=4, name="t1")
                nc.vector.tensor_mul(t1[:], sum_ps[:], mean_sb[:])
                t2 = sp.tile([1, R], F32, tag="stat", bufs=4, name="t2")
                nc.vector.tensor_sub(t2[:], ssq_ps[:], t1[:])
                # std = sqrt(t2/(D-1)); denom = std + eps; inv = 1/denom
                std = sp.tile([1, R], F32, tag="stat", bufs=4, name="std")
                nc.scalar.activation(std[:], t2[:], AF.Sqrt, scale=1.0 / (D - 1))
                se = sp.tile([1, R], F32, tag="stat", bufs=4, name="se")
                nc.vector.tensor_scalar_add(se[:], std[:], EPS)
                inv = sp.tile([1, R], F32, tag="stat", bufs=4, name="inv")
                nc.vector.reciprocal_approx_fast(inv[:], se[:])
                m_b = sp.tile([P, R], F32, tag="bcast", bufs=4, name="m_b")
                nc.gpsimd.partition_broadcast(m_b[:], mean_sb[:], channels=P)
                inv_b = sp.tile([P, R], F32, tag="bcast", bufs=4, name="inv_b")
                nc.gpsimd.partition_broadcast(inv_b[:], inv[:], channels=P)
                outs = []
                for t in range(NT):
                    tt = sp.tile([P, R], F32, tag="tmp", bufs=3, name="tt")
                    nc.vector.tensor_sub(tt[:], xts[t][:], m_b[:])
                    tt2 = sp.tile([P, R], F32, tag="tmp", bufs=3, name="tt2")
                    nc.vector.scalar_tensor_tensor(
                        tt2[:], tt[:], a_col[:, t:t + 1], inv_b[:],
                        op0=OP.mult, op1=OP.mult,
                    )
                    o = sp.tile([P, R], out_dtype, tag=out_tag, bufs=out_bufs,
                                name="lnout")
                    nc.vector.tensor_scalar_add(o[:], tt2[:], b_col[:, t:t + 1])
                    outs.append(o)
                return outs

            def proj_w_stationary(h_tiles, w_big, bias_col, out_tag,
                                  out_bufs, out_dtype=BF16):
                """out[dout, rows] (transposed): lhsT = W[k-tile, m-tile]."""
                outs = []
                for m in range(NT):
                    ps = pp.tile([P, R], F32, tag="mm", bufs=2, name="proj_ps")
                    for k in range(NT):
                        nc.tensor.matmul(
                            ps[:],
                            w_big[:, k * D + m * P: k * D + (m + 1) * P],
                            h_tiles[k][:], start=(k == 0), stop=(k == NT - 1),
                        )
                    o = sp.tile([P, R], out_dtype, tag=out_tag, bufs=out_bufs,
                                name="proj_o")
                    nc.vector.tensor_scalar_add(o[:], ps[:], bias_col[:, m:m + 1])
                    outs.append(o)
                return outs

            # ---- the 8 layers --------------------------------------------
            for l in range(NL):
                # per-layer bias / LN param columns
                bq_c = load_cols("bq_c", bq_d.ap()[l], D)
                bk_c = load_cols("bk_c", bk_d.ap()[l], D)
                bo_c = load_cols("bo_c", bo_d.ap()[l], D)
                b1_c = load_cols("b1_c", b1_d.ap()[l], DFF)
                b2_c = load_cols("b2_c", b2_d.ap()[l], D)
                l1a_c = load_cols("l1a_c", l1a_d.ap()[l], D)
                l1b_c = load_cols("l1b_c", l1b_d.ap()[l], D)
                l2a_c = load_cols("l2a_c", l2a_d.ap()[l], D)
                l2b_c = load_cols("l2b_c", l2b_d.ap()[l], D)
                bv_row = sp.tile([1, D], F32, tag="bvrow", bufs=2, name="bv_row")
                nc.sync.dma_start(
                    bv_row[:], bv_d.ap()[l].rearrange("(o d) -> o d", o=1)
                )
                bv_rowb = sp.tile([1, D], BF16, tag="bvrow", bufs=2,
                                  name="bv_rowb")
                nc.vector.tensor_copy(bv_rowb[:], bv_row[:])

                # weight tiles: one big SBUF tile per tensor per layer,
                # free dim = (ktile, dout): w?_sb[:, k*W + m] = W[l, k*P+p, m]
                def load_w(name, tag, src, width):
                    nkt = src.shape[1] // P
                    wt_ = sp.tile([P, nkt * width], BF16, tag=tag, bufs=1,
                                  name=name)
                    nc.sync.dma_start(
                        wt_[:].rearrange("p (k m) -> p k m", m=width),
                        src.ap()[l].rearrange(
                            "(k p) m -> k p m", p=P
                        ).rearrange("k p m -> p k m"),
                    )
                    return wt_

                wq_sb = load_w("wq_sb", "wq", wq_d, D)
                wk_sb = load_w("wk_sb", "wk", wk_d, D)
                wv_sb = load_w("wv_sb", "wv", wv_d, D)
                wo_sb = load_w("wo_sb", "wo", wo_d, D)
                w1_sb = load_w("w1_sb", "w1", w1_d, DFF)
                w2_sb = load_w("w2_sb", "w2", w2_d, D)

                # LN1 -> h1 (bf16, feature-major)
                h1 = layernorm(xt, l1a_c, l1b_c, BF16, "h1", 4)

                # K projection (transposed) and V projection (row-major,
                # with per-head ones column) -> AllGather bounce
                agin = dp.tile([2 * R, VW], BF16, tag="agin", bufs=2,
                               name="agin")
                kt_sb = proj_w_stationary(h1, wk_sb, bk_c, "kt", 4)
                for t in range(NT):
                    nc.sync.dma_start(
                        agin[t * P:(t + 1) * P, 0:D], kt_sb[t][:]
                    )
                for rt in range(NT):
                    vps = pp.tile([P, D], F32, tag="mm", bufs=2, name="vps")
                    nc.tensor.matmul(
                        vps[:], ones_row_bf[:, 0:P], bv_rowb[:],
                        start=True, stop=False,
                    )
                    for k in range(NT):
                        nc.tensor.matmul(
                            vps[:], h1[k][:, rt * P:(rt + 1) * P],
                            wv_sb[:, k * D:(k + 1) * D],
                            start=False, stop=(k == NT - 1),
                        )
                    v_sb = sp.tile([P, VW], BF16, tag="v_sb", bufs=3,
                                   name="v_sb")
                    nc.vector.memset(v_sb[:], 1.0)
                    nc.scalar.activation(
                        v_sb[:].rearrange("p (h w) -> p h w", w=DK + 1)[:, :, 0:DK],
                        vps[:].rearrange("p (h w) -> p h w", w=DK),
                        AF.Copy,
                    )
                    nc.sync.dma_start(
                        agin[R + rt * P:R + (rt + 1) * P, :], v_sb[:]
                    )
                agout = dp.tile([NCORES * 2 * R, VW], BF16, tag="agout",
                                bufs=2, addr_space="Shared", name="agout")
                nc.gpsimd.collective_compute(
                    "AllGather", OP.bypass,
                    replica_groups=[list(range(NCORES))],
                    ins=[agin.opt()], outs=[agout.opt()],
                )

                # Q projection (overlaps the AllGather)
                qt_sb = proj_w_stationary(h1, wq_sb, bq_c, "qt", 4)

                # load gathered K into SBUF (resident for the layer):
                # kreg[rc][p, t*R + rr] = Kt_rc[t*P+p, rr]
                ag4 = agout[:].rearrange(
                    "(rc x p) c -> rc x p c", rc=NCORES, p=P
                )  # x: 0-3 = Kt tiles, 4-7 = V batch rows
                kreg = []
                for rc in range(NCORES):
                    kr = sp.tile([P, NT * R], BF16, tag="kreg", bufs=8,
                                 name="kr")
                    nc.sync.dma_start(
                        kr[:].rearrange("p (t c) -> p t c", c=R),
                        ag4[rc, 0:NT, :, 0:R].rearrange("t p c -> p t c"),
                    )
                    kreg.append(kr)

                # attention: batch-outer (V streams per batch), heads in
                # pairs (even head on PE rows 0-63, odd on 64-127 ->
                # concurrent sub-array matmuls), scores split in two
                # 512-wide PSUM half-tiles so exp/AV overlap next scores.
                atn = [sp.tile([P, R], BF16, tag="atn", bufs=4, name="atn")
                       for _ in range(NT)]
                NH = NCORES // 2  # key-chunks per half
                for b in range(B):
                    vreg = sp.tile([P, NCORES * VW], BF16, tag="vreg", bufs=2,
                                   name="vreg")
                    nc.sync.dma_start(
                        vreg[:].rearrange("p (rc c) -> p rc c", c=VW),
                        ag4[0:NCORES, NT + b, :, :].rearrange(
                            "rc p c -> p rc c"
                        ),
                    )
                    for hp in range(H // 2):
                        sc = {}
                        for half in range(2):
                            for par in range(2):
                                s_ps = pp.tile([P, NH * P], F32, tag="scores",
                                               bufs=4, name="s_ps")
                                sc[(half, par)] = s_ps
                        probs2 = [
                            sp.tile([P, S], BF16, tag="probs", bufs=4,
                                    name="probs")
                            for _ in range(2)
                        ]
                        for half in range(2):
                            for kc in range(NH * half, NH * (half + 1)):
                                for par in range(2):
                                    o_h = par * DK
                                    nc.tensor.matmul(
                                        sc[(half, par)][:, (kc - NH * half) * P:
                                                        (kc - NH * half + 1) * P],
                                        kreg[kc][o_h:o_h + DK,
                                                 hp * R + b * SC:
                                                 hp * R + (b + 1) * SC],
                                        qt_sb[hp][o_h:o_h + DK,
                                                  b * SC:(b + 1) * SC],
                                        start=True, stop=True,
                                    )
                            for par in range(2):
                                nc.scalar.activation(
                                    probs2[par][:, half * NH * P:
                                                (half + 1) * NH * P],
                                    sc[(half, par)][:], AF.Exp,
                                    scale=1.0 / math.sqrt(DK),
                                )
                                nc.vector.tensor_mul(
                                    probs2[par][:, half * NH * P:
                                                (half + 1) * NH * P],
                                    probs2[par][:, half * NH * P:
                                                (half + 1) * NH * P],
                                    mask_sb[:, half * NH * P:
                                            (half + 1) * NH * P],
                                )
                        for par in range(2):
                            h = 2 * hp + par
                            o_h = par * DK
                            at_ps = pp.tile([DK + 1, SC], F32, tag="small",
                                            bufs=2, name="at_ps")
                            for kc in range(NCORES):
                                nc.tensor.matmul(
                                    at_ps[:],
                                    vreg[:, kc * VW + h * (DK + 1):
                                         kc * VW + (h + 1) * (DK + 1)],
                                    probs2[par][:, kc * P:(kc + 1) * P],
                                    start=(kc == 0), stop=(kc == NCORES - 1),
                                )
                            recip = sp.tile([1, SC], F32, tag="stat", bufs=4,
                                            name="recip")
                            rs_sb = sp.tile([1, SC], F32, tag="stat",
                                            bufs=4, name="rs_sb")
                            nc.vector.tensor_copy(rs_sb[:], at_ps[DK:DK + 1, :])
                            nc.vector.reciprocal_approx_fast(recip[:], rs_sb[:])
                            rb_sb = sp.tile([DK, SC], F32, tag="rb", bufs=3,
                                            name="rb_sb")
                            nc.gpsimd.partition_broadcast(
                                rb_sb[:], recip[:], channels=DK
                            )
                            nc.vector.tensor_mul(
                                atn[hp][o_h:o_h + DK, b * SC:(b + 1) * SC],
                                at_ps[0:DK, :], rb_sb[:],
                            )

                # Wo projection (transposed out) + residual
                xt2 = []
                for m in range(NT):
                    ops = pp.tile([P, R], F32, tag="mm", bufs=2, name="ops")
                    for k in range(NT):
                        nc.tensor.matmul(
                            ops[:],
                            wo_sb[:, k * D + m * P: k * D + (m + 1) * P],
                            atn[k][:],
                            start=(k == 0), stop=(k == NT - 1),
                        )
                    xn = sp.tile([P, R], F32, tag="xt", bufs=8, name="xn")
                    nc.vector.scalar_tensor_tensor(
                        xn[:], ops[:], bo_c[:, m:m + 1], xt[m][:],
                        op0=OP.add, op1=OP.add,
                    )
                    xt2.append(xn)

                # LN2 -> h2; FFN
                h2 = layernorm(xt2, l2a_c, l2b_c, BF16, "h1", 4)
                zt = []
                for mt in range(DFF // P):
                    zps = pp.tile([P, R], F32, tag="mm", bufs=2, name="zps")
                    for k in range(NT):
                        nc.tensor.matmul(
                            zps[:],
                            w1_sb[:, k * DFF + mt * P: k * DFF + (mt + 1) * P],
                            h2[k][:],
                            start=(k == 0), stop=(k == NT - 1),
                        )
                    z = sp.tile([P, R], BF16, tag="zt", bufs=17, name="z")
                    nc.scalar.activation(
                        z[:], zps[:], AF.Relu, bias=b1_c[:, mt:mt + 1]
                    )
                    zt.append(z)
                xt3 = []
                for m in range(NT):
                    fps = pp.tile([P, R], F32, tag="mm", bufs=2, name="fps")
                    for k in range(DFF // P):
                        nc.tensor.matmul(
                            fps[:],
                            w2_sb[:, k * D + m * P: k * D + (m + 1) * P],
                            zt[k][:],
                            start=(k == 0), stop=(k == DFF // P - 1),
                        )
                    xn2 = sp.tile([P, R], F32, tag="xt", bufs=8, name="xn2")
                    nc.vector.scalar_tensor_tensor(
                        xn2[:], fps[:], b2_c[:, m:m + 1], xt2[m][:],
                        op0=OP.add, op1=OP.add,
                    )
                    xt3.append(xn2)
                xt = xt3

            # ---- final LN + output ---------------------------------------
            yt = layernorm(xt, fa_col, fb_col, F32, "yt", 5)
            for t in range(NT):
                nc.sync.dma_start(y_d.ap()[t * P:(t + 1) * P, :], yt[t][:])

    nc.compile()
    return nc


_CACHED = {}


def _get_program():
    if "nc" not in _CACHED:
        _CACHED["nc"] = _build()
    return _CACHED["nc"]


def _pe_table():
    pos = np.arange(S, dtype=np.float32)[:, None]
    div = np.exp(
        np.arange(0, D, 2, dtype=np.float32) * (-math.log(10000.0) / D)
    )
    pe = np.zeros((S, D), np.float32)
    pe[:, 0::2] = np.sin(pos * div)
    pe[:, 1::2] = np.cos(pos * div)
    return pe


def _make_in_maps(inputs):
    f32 = lambda a: np.ascontiguousarray(np.asarray(a), dtype=np.float32)
    bf = lambda a: np.ascontiguousarray(np.asarray(a)).astype(BF16NP)

    x = f32(inputs["x"])                      # (B, S, D)
    pe = _pe_table()
    shared = {
        "wq": bf(inputs["Wq"]), "wk": bf(inputs["Wk"]),
        "wv": bf(inputs["Wv"]), "wo": bf(inputs["Wo"]),
        "w1": bf(inputs["W1"]), "w2": bf(inputs["W2"]),
        "bq": f32(inputs["bq"]), "bk": f32(inputs["bk"]),
        "bv": f32(inputs["bv"]), "bo": f32(inputs["bo"]),
        "b1": f32(inputs["b1"]), "b2": f32(inputs["b2"]),
        "l1a": f32(inputs["ln1_a"]), "l1b": f32(inputs["ln1_b"]),
        "l2a": f32(inputs["ln2_a"]), "l2b": f32(inputs["ln2_b"]),
        "fa": f32(inputs["fa"]), "fb": f32(inputs["fb"]),
    }
    in_maps = []
    for r in range(NCORES):
        sl = slice(r * SC, (r + 1) * SC)
        # [D, B*SC] feature-major slice, rows b-major
        x_t = np.ascontiguousarray(
            x[:, sl, :].transpose(2, 0, 1).reshape(D, R)
        )
        pe_t = np.ascontiguousarray(pe[sl, :].T)
        # mask[k, kc*128+q] = 1 if key (kc*128+k) is visible to query (r*128+q)
        kglob = (np.arange(NCORES)[:, None, None] * P
                 + np.arange(P)[None, :, None])          # [kc, k, 1]
        qglob = r * P + np.arange(P)[None, None, :]       # [1, 1, q]
        m = (kglob <= qglob)                              # [kc, k, q]
        mask = np.ascontiguousarray(
            m.transpose(1, 0, 2).reshape(P, S)
        ).astype(BF16NP)
        in_maps.append({"x_t": x_t, "pe_t": pe_t, "mask": mask, **shared})
    return in_maps


def kernel(**inputs):
    nc = _get_program()
    in_maps = _make_in_maps(inputs)
    res = bass_utils.run_bass_kernel_spmd(
        nc, in_maps, core_ids=list(range(NCORES))
    )
    out = np.empty((B, S, D), np.float32)
    for r in range(NCORES):
        y_t = res.results[r]["y_t"]                       # [D, R]
        out[:, r * SC:(r + 1) * SC, :] = (
            y_t.reshape(D, B, SC).transpose(1, 2, 0)
        )
    return out


if __name__ == "__main__":
    rng = np.random.default_rng(0)
    ins = {"x": rng.standard_normal((B, S, D)).astype(np.float32)}
    print(kernel(**ins).shape)
